# revision 7
# baseline (speedup 1.0000x reference)
"""Trainium2 Bass kernel for nn_ExemplarNoAttention (retrieval_knn).

logits[b,c] = log(eps + sum_{e: label[e]==c} exp(-beta * ||x_b - E_e||^2))

Sharding: exemplar bank Ne=50000 split across 8 NeuronCores (one SPMD
program; per-core exemplar slabs). Host does O(Ne*d) layout prep: reorder
exemplars so every core's slab is grouped by class with identical per-class
segment sizes (padded slots contribute exactly 0).

Device pipeline per core and per batch-tile of 128 queries:
  TensorE : psum[b,e] = 2*beta*<x_b,E_e> - beta*e2_e   in two matmuls per
            512-chunk: a K=64 feature matmul + a K=1 "e2 row" matmul that
            accumulates into the same PSUM bank. Consecutive chunks use
            alternating 64-row halves of the PE array (the weight data is
            duplicated in both halves) so each LDWEIGHTS overlaps the
            in-flight matmul on the other half instead of serializing.
  ScalarE : for most classes: one Exp activation per class segment with
            fused accumulation (accum_out) -> class sum directly (f32).
  VectorE : remaining classes: ScalarE writes exp() to bf16 SBUF in one big
            instruction, VectorE does tensor_scalar-with-accum per segment.
            (load-balance knob N_ACT_CLASSES)
  One AllReduce of the (128, 8*10) partial class sums across the 8 cores.
  ScalarE : logits = Ln(class_sums + eps); host reshapes (128,80)->(1024,10).
"""

import os
import numpy as np
import ml_dtypes

NUM_CLASSES = 10
EPS = 1e-12
N_CORES = 8
B = 1024
D = 64
NE = 50000
BT = 128
NBT = B // BT
SEG_ALIGN = 32
CHUNK = 512
WIN = 2048            # max psum window (4 banks)
N_ACT_CLASSES = 3     # classes reduced via fused Exp+accum on ScalarE

LAST_EXEC_NS = None
LAST_RESULTS = None
TRACE = bool(int(os.environ.get("KERNEL_TRACE", "0")))
TRACE_DIR = os.environ.get("KERNEL_TRACE_DIR", "")


def _host_prep(x, exemplars, exemplar_labels, beta_raw):
    x = np.asarray(x, dtype=np.float32)
    E = np.asarray(exemplars, dtype=np.float32)
    labels = np.asarray(exemplar_labels).astype(np.int64)
    beta = float(np.logaddexp(0.0, np.float64(beta_raw.reshape(-1)[0])))

    per_core_idx = [[None] * NUM_CLASSES for _ in range(N_CORES)]
    max_cc = np.zeros(NUM_CLASSES, dtype=np.int64)
    for c in range(NUM_CLASSES):
        idx_c = np.nonzero(labels == c)[0]
        n = len(idx_c)
        base, rem = divmod(n, N_CORES)
        sizes = [base + (1 if i < rem else 0) for i in range(N_CORES)]
        off = 0
        for i in range(N_CORES):
            per_core_idx[i][c] = idx_c[off:off + sizes[i]]
            off += sizes[i]
        max_cc[c] = max(sizes) if n else 0

    seg_sizes = [max(SEG_ALIGN, int(-(-m // SEG_ALIGN) * SEG_ALIGN)) for m in max_cc]
    seg_offs = np.concatenate([[0], np.cumsum(seg_sizes)]).astype(np.int64)
    e_pad = int(seg_offs[-1])

    e2 = (E.astype(np.float64) ** 2).sum(axis=1)
    ea_cores = []
    e2_cores = []
    for i in range(N_CORES):
        ea = np.zeros((D, e_pad), dtype=np.float32)
        e2row = np.full((e_pad,), -1.0e38, dtype=np.float32)  # pads -> exp()==0
        for c in range(NUM_CLASSES):
            idx = per_core_idx[i][c]
            o = int(seg_offs[c])
            if len(idx):
                ea[:, o:o + len(idx)] = (2.0 * beta) * E[idx].T
                e2row[o:o + len(idx)] = (-beta * e2[idx]).astype(np.float32)
        ea_cores.append(
            np.concatenate([ea, ea], axis=0).astype(ml_dtypes.bfloat16)
        )
        e2_cores.append(
            np.stack([e2row, e2row], axis=0).astype(ml_dtypes.bfloat16)
        )

    xa = np.concatenate([x.T, x.T], axis=0).astype(ml_dtypes.bfloat16)  # (128,B)
    x2 = (x.astype(np.float64) ** 2).sum(axis=1)
    bias = (-beta * x2).astype(np.float32).reshape(NBT, BT).T.copy()

    return ea_cores, e2_cores, xa, bias, seg_offs, seg_sizes, e_pad


def _plan_windows(seg_sizes):
    """Group whole classes into psum windows of <= WIN columns."""
    wins = []  # list of (class_lo, class_hi, col_off, col_len)
    c = 0
    off = 0
    while c < NUM_CLASSES:
        lo = c
        ln = 0
        while c < NUM_CLASSES and ln + seg_sizes[c] <= WIN:
            ln += seg_sizes[c]
            c += 1
        wins.append((lo, c, off, ln))
        off += ln
    return wins


def _build_program(seg_offs, seg_sizes, e_pad):
    from contextlib import ExitStack
    import concourse.bass as bass
    import concourse.tile as tile
    from concourse import bacc, mybir

    f32 = mybir.dt.float32
    bf16 = mybir.dt.bfloat16

    nc = bacc.Bacc(
        "TRN2",
        target_bir_lowering=False,
        debug=False,
        enable_asserts=False,
        num_devices=N_CORES,
    )

    ea_d = nc.dram_tensor("ea", [2 * D, e_pad], bf16, kind="ExternalInput").ap()
    e2_d = nc.dram_tensor("e2", [2, e_pad], bf16, kind="ExternalInput").ap()
    xa_d = nc.dram_tensor("xa", [2 * D, B], bf16, kind="ExternalInput").ap()
    bias_d = nc.dram_tensor("biasx", [BT, NBT], f32, kind="ExternalInput").ap()
    out_d = nc.dram_tensor(
        "logits", [BT, NBT * NUM_CLASSES], f32, kind="ExternalOutput"
    ).ap()

    wins = _plan_windows(seg_sizes)
    max_seg = max(seg_sizes)
    dve_start = int(seg_offs[N_ACT_CLASSES])     # DVE classes are the tail
    dve_len = e_pad - dve_start

    with tile.TileContext(nc) as tc, ExitStack() as ctx:
        const_pool = ctx.enter_context(tc.tile_pool(name="const", bufs=1))
        psum_pool = ctx.enter_context(tc.tile_pool(name="psum", bufs=2, space="PSUM"))
        sims_pool = ctx.enter_context(tc.tile_pool(name="sims", bufs=2))
        cls_pool = ctx.enter_context(tc.tile_pool(name="cls", bufs=3))
        junk_pool = ctx.enter_context(tc.tile_pool(name="junk", bufs=2))
        res_pool = ctx.enter_context(tc.tile_pool(name="res", bufs=1))
        dram_pool = ctx.enter_context(tc.tile_pool(name="dram", bufs=1, space="DRAM"))

        xa_t = const_pool.tile([2 * D, B], bf16, name="xa_t")
        nc.sync.dma_start(out=xa_t[:], in_=xa_d[:])
        bias_t = const_pool.tile([BT, NBT], f32, name="bias_t")
        nc.sync.dma_start(out=bias_t[:], in_=bias_d[:])
        eps_t = const_pool.tile([BT, 1], f32, name="eps_t")
        nc.vector.memset(eps_t[:], float(EPS))
        ones_t = const_pool.tile([D + 1, BT], bf16, name="ones_t")
        nc.vector.memset(ones_t[0:1, :], 1.0)
        nc.vector.memset(ones_t[D:D + 1, :], 1.0)
        e2_t = const_pool.tile([D + 1, e_pad], bf16, name="e2_t")
        nc.sync.dma_start(out=e2_t[0:1, :], in_=e2_d[0:1, :])
        nc.sync.dma_start(out=e2_t[D:D + 1, :], in_=e2_d[1:2, :])

        ea_w = []
        for (clo, chi, woff, wlen) in wins:
            t_ = const_pool.tile([2 * D, wlen], bf16, name=f"ea_w{woff}")
            half = (wlen + CHUNK - 1) // CHUNK // 2 * CHUNK
            if half == 0 or half >= wlen:
                nc.sync.dma_start(out=t_[:], in_=ea_d[:, woff:woff + wlen])
            else:
                nc.sync.dma_start(out=t_[:, :half], in_=ea_d[:, woff:woff + half])
                nc.sync.dma_start(
                    out=t_[:, half:], in_=ea_d[:, woff + half:woff + wlen]
                )
            ea_w.append(t_)

        bnc_in = dram_pool.tile([BT, NBT * NUM_CLASSES], f32, name="bnc_in")
        bnc_out = dram_pool.tile(
            [BT, NBT * NUM_CLASSES], f32, name="bnc_out", addr_space="Shared"
        )

        kchunk = 0  # global chunk counter -> alternating PE row half
        for t in range(NBT):
            cls = cls_pool.tile([BT, NUM_CLASSES], f32, tag="cls")
            sims = None
            if dve_len:
                sims = sims_pool.tile([BT, dve_len], bf16, tag="sims", name="sims")
            junk = junk_pool.tile([BT, max_seg], bf16, tag="junk")
            for wi, (clo, chi, woff, wlen) in enumerate(wins):
                ps = psum_pool.tile([BT, WIN], f32, tag="ps")
                co = 0
                while co < wlen:
                    cl = min(CHUNK, wlen - co)
                    h = (kchunk % 2) * D
                    kchunk += 1
                    nc.tensor.matmul(
                        ps[:, co:co + cl],
                        lhsT=xa_t[h:h + D, t * BT:(t + 1) * BT],
                        rhs=ea_w[wi][h:h + D, co:co + cl],
                        start=True,
                        stop=False,
                    )
                    nc.tensor.matmul(
                        ps[:, co:co + cl],
                        lhsT=ones_t[h:h + 1, :],
                        rhs=e2_t[h:h + 1, woff + co:woff + co + cl],
                        start=False,
                        stop=True,
                    )
                    co += cl
                # per-class fused exp+accumulate on ScalarE
                for c in range(clo, min(chi, N_ACT_CLASSES)):
                    so = int(seg_offs[c]) - woff
                    s = seg_sizes[c]
                    nc.scalar.activation(
                        junk[:, :s],
                        ps[:, so:so + s],
                        mybir.ActivationFunctionType.Exp,
                        bias=bias_t[:, t:t + 1],
                        scale=1.0,
                        accum_out=cls[:, c:c + 1],
                    )
                # remaining classes: bulk exp -> bf16 sims for VectorE
                dlo = max(clo, N_ACT_CLASSES)
                if dlo < chi:
                    so = int(seg_offs[dlo]) - woff
                    ln = (int(seg_offs[chi]) - woff) - so
                    nc.scalar.activation(
                        sims[:, int(seg_offs[dlo]) - dve_start:
                             int(seg_offs[dlo]) - dve_start + ln],
                        ps[:, so:so + ln],
                        mybir.ActivationFunctionType.Exp,
                        bias=bias_t[:, t:t + 1],
                        scale=1.0,
                    )
            for c in range(N_ACT_CLASSES, NUM_CLASSES):
                o = int(seg_offs[c]) - dve_start
                s = seg_sizes[c]
                nc.vector.tensor_scalar(
                    junk[:, :s],
                    sims[:, o:o + s],
                    1.0,
                    None,
                    mybir.AluOpType.mult,
                    mybir.AluOpType.add,
                    accum_out=cls[:, c:c + 1],
                )
            nc.sync.dma_start(
                out=bnc_in[:, t * NUM_CLASSES:(t + 1) * NUM_CLASSES], in_=cls[:]
            )

        nc.gpsimd.collective_compute(
            "AllReduce",
            mybir.AluOpType.add,
            replica_groups=[list(range(N_CORES))],
            ins=[bnc_in[:].opt()],
            outs=[bnc_out[:].opt()],
        )
        res = res_pool.tile([BT, NBT * NUM_CLASSES], f32, name="res")
        nc.sync.dma_start(out=res[:], in_=bnc_out[:])
        logit = res_pool.tile([BT, NBT * NUM_CLASSES], f32, name="logit")
        nc.scalar.activation(
            logit[:],
            res[:],
            mybir.ActivationFunctionType.Ln,
            bias=eps_t[:, 0:1],
            scale=1.0,
        )
        nc.sync.dma_start(out=out_d, in_=logit[:])

    nc.compile()
    return nc


def kernel(x, exemplars, exemplar_labels, beta_raw):
    global LAST_EXEC_NS, LAST_RESULTS
    from concourse.bass_utils import run_bass_kernel_spmd

    ea_cores, e2_cores, xa, bias, seg_offs, seg_sizes, e_pad = _host_prep(
        x, exemplars, exemplar_labels, beta_raw
    )
    nc = _build_program(seg_offs, seg_sizes, e_pad)

    in_maps = [
        {"ea": ea_cores[i], "e2": e2_cores[i], "xa": xa, "biasx": bias}
        for i in range(N_CORES)
    ]
    kwargs = {}
    if TRACE:
        kwargs["trace"] = True
        if TRACE_DIR:
            os.makedirs(TRACE_DIR, exist_ok=True)
            kwargs["tmpdir"] = TRACE_DIR
    ret = run_bass_kernel_spmd(nc, in_maps, list(range(N_CORES)), **kwargs)
    LAST_EXEC_NS = ret.exec_time_ns
    LAST_RESULTS = ret
    flat = np.asarray(ret.results[0]["logits"], dtype=np.float32)  # (128, 80)
    out = flat.reshape(BT, NBT, NUM_CLASSES).transpose(1, 0, 2).reshape(
        B, NUM_CLASSES
    )
    return np.ascontiguousarray(out)


# revision 8
# speedup vs baseline: 1.9032x; 1.9032x over previous
"""Trainium2 Bass kernel for nn_ExemplarNoAttention (retrieval_knn).

logits[b,c] = log(eps + sum_{e: label[e]==c} exp(-beta * ||x_b - E_e||^2))

Sharding: exemplar bank Ne=50000 split across 8 NeuronCores (one SPMD
program; per-core exemplar slabs). Host does O(Ne*d) layout prep: reorder
exemplars so every core's slab is grouped by class with identical per-class
segment sizes (padded slots contribute exactly 0 via a -1e38 bias column).

Device pipeline per core, per batch-tile of 128 queries:
  TensorE : psum[b,e] = 2*beta*<x_b,E_e> - beta*e2_e  (bf16 GEMM, K=65:
            rows 0..63 features, row 64 = (1 -> -beta*e2) augmentation)
  ScalarE : n_A classes: one Exp per class segment with fused accum_out
            -> f32 class sum directly. Remaining classes: bulk Exp -> bf16.
  VectorE : remaining classes: tensor_scalar with fused accum per segment.
  Partial class sums cast to bf16; two AllReduce halves (batch-tiles 0-3
  overlap the second half of compute).
  ScalarE : logits = Ln(sums + eps); host reshapes (128,80)->(1024,10).
"""

import os
import numpy as np
import ml_dtypes

NUM_CLASSES = 10
EPS = 1e-12
N_CORES = 8
B = 1024
D = 64
NE = 50000
BT = 128
NBT = B // BT
SEG_ALIGN = 32
CHUNK = 512
WIN = 2048            # max psum window (4 banks)
N_ACT_CLASSES = 3     # classes reduced via fused Exp+accum on ScalarE

LAST_EXEC_NS = None
LAST_RESULTS = None
TRACE = bool(int(os.environ.get("KERNEL_TRACE", "0")))
TRACE_DIR = os.environ.get("KERNEL_TRACE_DIR", "")


def _host_prep(x, exemplars, exemplar_labels, beta_raw):
    x = np.asarray(x, dtype=np.float32)
    E = np.asarray(exemplars, dtype=np.float32)
    labels = np.asarray(exemplar_labels).astype(np.int64)
    beta = float(np.logaddexp(0.0, np.float64(beta_raw.reshape(-1)[0])))

    per_core_idx = [[None] * NUM_CLASSES for _ in range(N_CORES)]
    max_cc = np.zeros(NUM_CLASSES, dtype=np.int64)
    for c in range(NUM_CLASSES):
        idx_c = np.nonzero(labels == c)[0]
        n = len(idx_c)
        base, rem = divmod(n, N_CORES)
        sizes = [base + (1 if i < rem else 0) for i in range(N_CORES)]
        off = 0
        for i in range(N_CORES):
            per_core_idx[i][c] = idx_c[off:off + sizes[i]]
            off += sizes[i]
        max_cc[c] = max(sizes) if n else 0

    seg_sizes = [max(SEG_ALIGN, int(-(-m // SEG_ALIGN) * SEG_ALIGN)) for m in max_cc]
    seg_offs = np.concatenate([[0], np.cumsum(seg_sizes)]).astype(np.int64)
    e_pad = int(seg_offs[-1])

    e2 = (E.astype(np.float64) ** 2).sum(axis=1)
    ea_cores = []
    for i in range(N_CORES):
        ea = np.zeros((D + 1, e_pad), dtype=np.float32)
        ea[D, :] = -1.0e38  # padded slots: exp() == 0
        for c in range(NUM_CLASSES):
            idx = per_core_idx[i][c]
            o = int(seg_offs[c])
            if len(idx):
                ea[:D, o:o + len(idx)] = (2.0 * beta) * E[idx].T
                ea[D, o:o + len(idx)] = (-beta * e2[idx]).astype(np.float32)
        ea_cores.append(ea.astype(ml_dtypes.bfloat16))

    xa = np.ones((D + 1, B), dtype=np.float32)
    xa[:D, :] = x.T
    xa = xa.astype(ml_dtypes.bfloat16)

    x2 = (x.astype(np.float64) ** 2).sum(axis=1)
    bias = (-beta * x2).astype(np.float32).reshape(NBT, BT).T.copy()

    return ea_cores, xa, bias, seg_offs, seg_sizes, e_pad


def _plan_windows(seg_sizes):
    wins = []  # (class_lo, class_hi, col_off, col_len)
    c = 0
    off = 0
    while c < NUM_CLASSES:
        lo = c
        ln = 0
        while c < NUM_CLASSES and ln + seg_sizes[c] <= WIN:
            ln += seg_sizes[c]
            c += 1
        wins.append((lo, c, off, ln))
        off += ln
    return wins


def _build_program(seg_offs, seg_sizes, e_pad):
    from contextlib import ExitStack
    import concourse.bass as bass
    import concourse.tile as tile
    from concourse import bacc, mybir

    f32 = mybir.dt.float32
    bf16 = mybir.dt.bfloat16

    nc = bacc.Bacc(
        "TRN2",
        target_bir_lowering=False,
        debug=False,
        enable_asserts=False,
        num_devices=N_CORES,
    )

    ea_d = nc.dram_tensor("ea", [D + 1, e_pad], bf16, kind="ExternalInput").ap()
    xa_d = nc.dram_tensor("xa", [D + 1, B], bf16, kind="ExternalInput").ap()
    bias_d = nc.dram_tensor("biasx", [BT, NBT], f32, kind="ExternalInput").ap()
    out_d = nc.dram_tensor(
        "logits", [BT, NBT * NUM_CLASSES], f32, kind="ExternalOutput"
    ).ap()

    wins = _plan_windows(seg_sizes)
    max_seg = max(seg_sizes)
    dve_start = int(seg_offs[N_ACT_CLASSES])
    dve_len = e_pad - dve_start
    NC2 = NUM_CLASSES * NBT // 2  # bf16 bounce half-width (40 cols)

    with tile.TileContext(nc) as tc, ExitStack() as ctx:
        const_pool = ctx.enter_context(tc.tile_pool(name="const", bufs=1))
        psum_pool = ctx.enter_context(tc.tile_pool(name="psum", bufs=2, space="PSUM"))
        sims_pool = ctx.enter_context(tc.tile_pool(name="sims", bufs=2))
        cls_pool = ctx.enter_context(tc.tile_pool(name="cls", bufs=3))
        junk_pool = ctx.enter_context(tc.tile_pool(name="junk", bufs=2))
        res_pool = ctx.enter_context(tc.tile_pool(name="res", bufs=1))
        dram_pool = ctx.enter_context(tc.tile_pool(name="dram", bufs=1, space="DRAM"))

        xa_t = const_pool.tile([D + 1, B], bf16, name="xa_t")
        nc.sync.dma_start(out=xa_t[:], in_=xa_d[:])
        bias_t = const_pool.tile([BT, NBT], f32, name="bias_t")
        nc.sync.dma_start(out=bias_t[:], in_=bias_d[:])
        eps_t = const_pool.tile([BT, 1], f32, name="eps_t")
        nc.vector.memset(eps_t[:], float(EPS))

        ea_w = []
        for (clo, chi, woff, wlen) in wins:
            t_ = const_pool.tile([D + 1, wlen], bf16, name=f"ea_w{woff}")
            nc.sync.dma_start(out=t_[:], in_=ea_d[:, woff:woff + wlen])
            ea_w.append(t_)

        # bf16 bounce halves for the two pipelined AllReduces
        bnc_in0 = dram_pool.tile([BT, NC2], bf16, name="bnc_in0")
        bnc_out0 = dram_pool.tile([BT, NC2], bf16, name="bnc_out0", addr_space="Shared")
        bnc_in1 = dram_pool.tile([BT, NC2], bf16, name="bnc_in1")
        bnc_out1 = dram_pool.tile([BT, NC2], bf16, name="bnc_out1", addr_space="Shared")

        cls_h = [None, None]  # bf16 gather tiles per half

        for t in range(NBT):
            half = t // (NBT // 2)
            if t % (NBT // 2) == 0:
                cls_h[half] = cls_pool.tile([BT, NC2], bf16, name=f"clsh{half}",
                                            tag=f"clsh{half}", bufs=1)
            cls = cls_pool.tile([BT, NUM_CLASSES], f32, tag="cls")
            sims = None
            if dve_len:
                sims = sims_pool.tile([BT, dve_len], bf16, tag="sims", name="sims")
            junk = junk_pool.tile([BT, max_seg], bf16, tag="junk")
            for wi, (clo, chi, woff, wlen) in enumerate(wins):
                ps = psum_pool.tile([BT, WIN], f32, tag="ps")
                co = 0
                while co < wlen:
                    cl = min(CHUNK, wlen - co)
                    nc.tensor.matmul(
                        ps[:, co:co + cl],
                        lhsT=xa_t[:, t * BT:(t + 1) * BT],
                        rhs=ea_w[wi][:, co:co + cl],
                        start=True,
                        stop=True,
                    )
                    co += cl
                for c in range(clo, min(chi, N_ACT_CLASSES)):
                    so = int(seg_offs[c]) - woff
                    s = seg_sizes[c]
                    nc.scalar.activation(
                        junk[:, :s],
                        ps[:, so:so + s],
                        mybir.ActivationFunctionType.Exp,
                        bias=bias_t[:, t:t + 1],
                        scale=1.0,
                        accum_out=cls[:, c:c + 1],
                    )
                dlo = max(clo, N_ACT_CLASSES)
                if dlo < chi:
                    so = int(seg_offs[dlo]) - woff
                    ln = (int(seg_offs[chi]) - woff) - so
                    do = int(seg_offs[dlo]) - dve_start
                    nc.scalar.activation(
                        sims[:, do:do + ln],
                        ps[:, so:so + ln],
                        mybir.ActivationFunctionType.Exp,
                        bias=bias_t[:, t:t + 1],
                        scale=1.0,
                    )
            for c in range(N_ACT_CLASSES, NUM_CLASSES):
                o = int(seg_offs[c]) - dve_start
                s = seg_sizes[c]
                nc.vector.tensor_scalar(
                    junk[:, :s],
                    sims[:, o:o + s],
                    1.0,
                    None,
                    mybir.AluOpType.mult,
                    mybir.AluOpType.add,
                    accum_out=cls[:, c:c + 1],
                )
            # cast this tile's f32 sums into the half's bf16 gather tile
            ti = t % (NBT // 2)
            nc.vector.tensor_copy(
                cls_h[half][:, ti * NUM_CLASSES:(ti + 1) * NUM_CLASSES], cls[:]
            )
            if t == NBT // 2 - 1:
                nc.sync.dma_start(out=bnc_in0[:], in_=cls_h[0][:])
                nc.gpsimd.collective_compute(
                    "AllReduce",
                    mybir.AluOpType.add,
                    replica_groups=[list(range(N_CORES))],
                    ins=[bnc_in0[:].opt()],
                    outs=[bnc_out0[:].opt()],
                )
            if t == NBT - 1:
                nc.sync.dma_start(out=bnc_in1[:], in_=cls_h[1][:])
                nc.gpsimd.collective_compute(
                    "AllReduce",
                    mybir.AluOpType.add,
                    replica_groups=[list(range(N_CORES))],
                    ins=[bnc_in1[:].opt()],
                    outs=[bnc_out1[:].opt()],
                )

        res = res_pool.tile([BT, NBT * NUM_CLASSES], bf16, name="res")
        nc.sync.dma_start(out=res[:, :NC2], in_=bnc_out0[:])
        nc.sync.dma_start(out=res[:, NC2:], in_=bnc_out1[:])
        logit = res_pool.tile([BT, NBT * NUM_CLASSES], f32, name="logit")
        nc.scalar.activation(
            logit[:],
            res[:],
            mybir.ActivationFunctionType.Ln,
            bias=eps_t[:, 0:1],
            scale=1.0,
        )
        nc.sync.dma_start(out=out_d, in_=logit[:])

    nc.compile()
    return nc


def kernel(x, exemplars, exemplar_labels, beta_raw):
    global LAST_EXEC_NS, LAST_RESULTS
    from concourse.bass_utils import run_bass_kernel_spmd

    ea_cores, xa, bias, seg_offs, seg_sizes, e_pad = _host_prep(
        x, exemplars, exemplar_labels, beta_raw
    )
    nc = _build_program(seg_offs, seg_sizes, e_pad)

    in_maps = [
        {"ea": ea_cores[i], "xa": xa, "biasx": bias} for i in range(N_CORES)
    ]
    kwargs = {}
    if TRACE:
        kwargs["trace"] = True
        if TRACE_DIR:
            os.makedirs(TRACE_DIR, exist_ok=True)
            kwargs["tmpdir"] = TRACE_DIR
    ret = run_bass_kernel_spmd(nc, in_maps, list(range(N_CORES)), **kwargs)
    LAST_EXEC_NS = ret.exec_time_ns
    LAST_RESULTS = ret
    flat = np.asarray(ret.results[0]["logits"], dtype=np.float32)  # (128, 80)
    out = flat.reshape(BT, NBT, NUM_CLASSES).transpose(1, 0, 2).reshape(
        B, NUM_CLASSES
    )
    return np.ascontiguousarray(out)


# revision 10
# speedup vs baseline: 3.0703x; 1.6132x over previous
"""Trainium2 Bass kernel for nn_ExemplarNoAttention (retrieval_knn).

logits[b,c] = log(eps + sum_{e: label[e]==c} exp(-beta * ||x_b - E_e||^2))

Sharding: data-parallel over the batch. Each of the 8 NeuronCores computes
its own 128 queries against the full exemplar bank (replicated, class-sorted
on the host); the host concatenates the per-core (128, 10) outputs. No
collectives: each core's pipeline is fully independent.

Device pipeline per core (one batch tile of 128 queries):
  TensorE : psum[b,e] = 2*beta*<x_b,E_e> - beta*e2_e  (bf16 GEMM, K=65:
            rows 0..63 = features, row 64 = 1 -> -beta*e2 augmentation row;
            one stationary weight load, 98 chunked matmuls at stream rate)
  ScalarE : exp(psum + bias_b) with bias_b = -beta*||x_b||^2. For segments
            of "ACT-route" classes the Exp carries a fused accum_out that
            yields the class-segment sum directly (f32). Other classes get
            bulk Exp into bf16 SBUF.
  VectorE : "DVE-route" class pieces: tensor_scalar with fused accumulate.
  Piece sums -> per-class sums (tiny reduce), logits = Ln(sums + eps),
  DMA out (128, 10) per core.
"""

import os
import numpy as np
import ml_dtypes

NUM_CLASSES = 10
EPS = 1e-12
N_CORES = 8
B = 1024
D = 64
NE = 50000
BT = 128
SEG_ALIGN = 32
CHUNK = 512
WIN = 2048             # psum window (4 banks)
N_ACT_CLASSES = 2      # classes whose sums come from fused Exp+accum (ScalarE)

LAST_EXEC_NS = None
LAST_RESULTS = None
TRACE = bool(int(os.environ.get("KERNEL_TRACE", "0")))
TRACE_DIR = os.environ.get("KERNEL_TRACE_DIR", "")


def _host_prep(x, exemplars, exemplar_labels, beta_raw):
    x = np.asarray(x, dtype=np.float32)
    E = np.asarray(exemplars, dtype=np.float32)
    labels = np.asarray(exemplar_labels).astype(np.int64)
    beta = float(np.logaddexp(0.0, np.float64(beta_raw.reshape(-1)[0])))

    # global class-sorted layout with 32-aligned per-class segments
    seg_idx = []
    seg_sizes = []
    for c in range(NUM_CLASSES):
        idx_c = np.nonzero(labels == c)[0]
        seg_idx.append(idx_c)
        seg_sizes.append(max(SEG_ALIGN, int(-(-len(idx_c) // SEG_ALIGN) * SEG_ALIGN)))
    seg_offs = np.concatenate([[0], np.cumsum(seg_sizes)]).astype(np.int64)
    e_pad = int(seg_offs[-1])

    e2 = (E.astype(np.float64) ** 2).sum(axis=1)
    ea = np.zeros((D + 1, e_pad), dtype=np.float32)
    ea[D, :] = -1.0e38  # padding slots contribute exp() == 0
    for c in range(NUM_CLASSES):
        idx = seg_idx[c]
        o = int(seg_offs[c])
        ea[:D, o:o + len(idx)] = (2.0 * beta) * E[idx].T
        ea[D, o:o + len(idx)] = (-beta * e2[idx]).astype(np.float32)
    ea = ea.astype(ml_dtypes.bfloat16)

    # per-core stationary x tiles and activation biases
    xa = np.ones((D + 1, B), dtype=np.float32)
    xa[:D, :] = x.T
    xa = xa.astype(ml_dtypes.bfloat16)
    x2 = (x.astype(np.float64) ** 2).sum(axis=1)
    bias = (-beta * x2).astype(np.float32)

    xa_cores = [np.ascontiguousarray(xa[:, i * BT:(i + 1) * BT]) for i in range(N_CORES)]
    bias_cores = [
        np.ascontiguousarray(bias[i * BT:(i + 1) * BT].reshape(BT, 1))
        for i in range(N_CORES)
    ]
    return ea, xa_cores, bias_cores, seg_offs, seg_sizes, e_pad


def _build_program(seg_offs, seg_sizes, e_pad):
    from contextlib import ExitStack
    import concourse.bass as bass
    import concourse.tile as tile
    from concourse import bacc, mybir

    f32 = mybir.dt.float32
    bf16 = mybir.dt.bfloat16

    nc = bacc.Bacc(
        "TRN2",
        target_bir_lowering=False,
        debug=False,
        enable_asserts=False,
        num_devices=N_CORES,
    )

    ea_d = nc.dram_tensor("ea", [D + 1, e_pad], bf16, kind="ExternalInput").ap()
    xa_d = nc.dram_tensor("xa", [D + 1, BT], bf16, kind="ExternalInput").ap()
    bias_d = nc.dram_tensor("biasx", [BT, 1], f32, kind="ExternalInput").ap()
    out_d = nc.dram_tensor("logits", [BT, NUM_CLASSES], f32, kind="ExternalOutput").ap()

    # windows of <= WIN columns; pieces = (class, window) intersections
    wins = []
    o = 0
    while o < e_pad:
        wins.append((o, min(WIN, e_pad - o)))
        o += WIN
    dve_start = int(seg_offs[N_ACT_CLASSES])
    dve_len = e_pad - dve_start

    # piece table per window: (class, col_off, col_len)
    win_pieces = []
    for (wo, wl) in wins:
        pieces = []
        for c in range(NUM_CLASSES):
            lo = max(int(seg_offs[c]), wo)
            hi = min(int(seg_offs[c + 1]), wo + wl)
            if lo < hi:
                pieces.append((c, lo, hi - lo))
        win_pieces.append(pieces)
    n_pieces_per_class = [0] * NUM_CLASSES
    piece_col = {}  # (c, lo) -> column in piece-sum tile
    pcol = 0
    for pieces in win_pieces:
        for (c, lo, ln) in pieces:
            piece_col[(c, lo)] = pcol
            n_pieces_per_class[c] += 1
            pcol += 1
    n_pieces = pcol
    class_piece_range = []
    acc = 0
    for c in range(NUM_CLASSES):
        class_piece_range.append((acc, acc + n_pieces_per_class[c]))
        acc += n_pieces_per_class[c]

    with tile.TileContext(nc) as tc, ExitStack() as ctx:
        const_pool = ctx.enter_context(tc.tile_pool(name="const", bufs=1))
        psum_pool = ctx.enter_context(tc.tile_pool(name="psum", bufs=2, space="PSUM"))
        sims_pool = ctx.enter_context(tc.tile_pool(name="sims", bufs=1))
        work_pool = ctx.enter_context(tc.tile_pool(name="work", bufs=1))
        junk_pool = ctx.enter_context(tc.tile_pool(name="junk", bufs=2))

        xa_t = const_pool.tile([D + 1, BT], bf16, name="xa_t")
        nc.sync.dma_start(out=xa_t[:], in_=xa_d[:])
        bias_t = const_pool.tile([BT, 1], f32, name="bias_t")
        nc.sync.dma_start(out=bias_t[:], in_=bias_d[:])
        eps_t = const_pool.tile([BT, 1], f32, name="eps_t")
        nc.vector.memset(eps_t[:], float(EPS))

        ea_w = []
        for (wo, wl) in wins:
            t_ = const_pool.tile([D + 1, wl], bf16, name=f"ea_w{wo}", tag=f"ea_w{wo}")
            nc.sync.dma_start(out=t_[:], in_=ea_d[:, wo:wo + wl])
            ea_w.append(t_)

        sims = sims_pool.tile([BT, dve_len], bf16, name="sims")
        pieces_t = work_pool.tile([BT, max(n_pieces, 1)], f32, name="pieces_t")
        junk = junk_pool.tile([BT, WIN], bf16, name="junkt")

        for wi, (wo, wl) in enumerate(wins):
            ps = psum_pool.tile([BT, WIN], f32, tag="ps")
            co = 0
            while co < wl:
                cl = min(CHUNK, wl - co)
                nc.tensor.matmul(
                    ps[:, co:co + cl],
                    lhsT=xa_t[:],
                    rhs=ea_w[wi][:, co:co + cl],
                    start=True,
                    stop=True,
                )
                co += cl
            # ACT-route pieces: fused exp + accumulate straight to piece sum
            for (c, lo, ln) in win_pieces[wi]:
                if c < N_ACT_CLASSES:
                    pc = piece_col[(c, lo)]
                    nc.scalar.activation(
                        junk[:, :ln],
                        ps[:, lo - wo:lo - wo + ln],
                        mybir.ActivationFunctionType.Exp,
                        bias=bias_t[:, 0:1],
                        scale=1.0,
                        accum_out=pieces_t[:, pc:pc + 1],
                    )
            # DVE-route region of this window: one bulk exp into bf16 sims
            dlo = max(wo, dve_start)
            if dlo < wo + wl:
                ln = wo + wl - dlo
                nc.scalar.activation(
                    sims[:, dlo - dve_start:dlo - dve_start + ln],
                    ps[:, dlo - wo:dlo - wo + ln],
                    mybir.ActivationFunctionType.Exp,
                    bias=bias_t[:, 0:1],
                    scale=1.0,
                )
            for (c, lo, ln) in win_pieces[wi]:
                if c >= N_ACT_CLASSES:
                    pc = piece_col[(c, lo)]
                    so = lo - dve_start
                    nc.vector.tensor_scalar(
                        junk[:, :ln],
                        sims[:, so:so + ln],
                        1.0,
                        None,
                        mybir.AluOpType.mult,
                        mybir.AluOpType.add,
                        accum_out=pieces_t[:, pc:pc + 1],
                    )

        # combine piece sums into class sums
        cls = work_pool.tile([BT, NUM_CLASSES], f32, name="clst")
        junkf = work_pool.tile([BT, max(n_pieces, 1)], f32, name="junkf")
        for c in range(NUM_CLASSES):
            plo, phi = class_piece_range[c]
            if phi - plo > 1:
                nc.vector.tensor_scalar(
                    junkf[:, plo:phi],
                    pieces_t[:, plo:phi],
                    1.0,
                    None,
                    mybir.AluOpType.mult,
                    mybir.AluOpType.add,
                    accum_out=cls[:, c:c + 1],
                )
            else:
                nc.vector.tensor_copy(cls[:, c:c + 1], pieces_t[:, plo:plo + 1])
        logit = work_pool.tile([BT, NUM_CLASSES], f32, name="logit")
        nc.scalar.activation(
            logit[:],
            cls[:],
            mybir.ActivationFunctionType.Ln,
            bias=eps_t[:, 0:1],
            scale=1.0,
        )
        nc.sync.dma_start(out=out_d, in_=logit[:])

    nc.compile()
    return nc


def kernel(x, exemplars, exemplar_labels, beta_raw):
    global LAST_EXEC_NS, LAST_RESULTS
    from concourse.bass_utils import run_bass_kernel_spmd

    ea, xa_cores, bias_cores, seg_offs, seg_sizes, e_pad = _host_prep(
        x, exemplars, exemplar_labels, beta_raw
    )
    nc = _build_program(seg_offs, seg_sizes, e_pad)

    in_maps = [
        {"ea": ea, "xa": xa_cores[i], "biasx": bias_cores[i]}
        for i in range(N_CORES)
    ]
    kwargs = {}
    if TRACE:
        kwargs["trace"] = True
        if TRACE_DIR:
            os.makedirs(TRACE_DIR, exist_ok=True)
            kwargs["tmpdir"] = TRACE_DIR
    ret = run_bass_kernel_spmd(nc, in_maps, list(range(N_CORES)), **kwargs)
    LAST_EXEC_NS = ret.exec_time_ns
    LAST_RESULTS = ret
    out = np.concatenate(
        [np.asarray(ret.results[i]["logits"], dtype=np.float32) for i in range(N_CORES)],
        axis=0,
    )
    return np.ascontiguousarray(out)
